# revision 1
# baseline (speedup 1.0000x reference)
"""MoEBertSelfAttention on 8 Trainium2 NeuronCores.

Strategy: data-parallel over batch (B=8 -> one batch element per core).
Each core computes its element's full self-attention:
    q = h @ Wq.T + bq ; k, v likewise
    S = q_h k_h^T / sqrt(dh) + mask ; P = softmax(S) * head_mask
    ctx = P v_h, heads concatenated.

On-device dataflow is fully transposed to avoid any on-chip transposes:
  - host passes H^T and W^T; projections produce Q^T/K^T (feature-major)
    and V in normal layout (token-major),
  - scores are computed as S^T (key position on partitions) so the additive
    attention mask is a per-partition bias on the exp() activation,
  - the softmax denominator rides as an extra all-ones column of V in the
    PV matmul; normalization uses a batched reciprocal (reshaped to all 128
    partitions via a DRAM bounce) and a partition-broadcast DMA,
  - host transposes the returned ctx^T back.
head_mask is folded into Wv/bv on the host (exact: probs*hm @ V == probs @ (hm*V)).
Matmuls run in float32r (full PE rate); PSUM/softmax stay fp32.

The next head-pair's Q/K projection matmuls are software-pipelined into the
current pair's attention loop as PE filler work so the tensor engine never
idles while the activation engine runs exp().
"""

import sys

if "/opt/trn_rl_repo" not in sys.path:
    sys.path.insert(0, "/opt/trn_rl_repo")

import numpy as np

import concourse.bacc as bacc
import concourse.bass as bass
import concourse.tile as tile
from concourse import mybir
from concourse.bass_utils import run_bass_kernel_spmd

S = 1024  # sequence length
D = 1024  # hidden size
H = 16  # heads
DH = 64  # head size
KT = D // 128  # 128-row tiles along a feature dim
NT = S // 512  # 512-col tiles along the sequence
HP = H // 2  # head pairs
N_CORES = 8

F32 = mybir.dt.float32
F32R = mybir.dt.float32r


def _ts(i, n):
    return slice(i * n, (i + 1) * n)


def build_program():
    nc = bacc.Bacc("TRN2", target_bir_lowering=False, debug=False, num_devices=N_CORES)

    hT = nc.dram_tensor("hT", [D, S], F32R, kind="ExternalInput").ap()
    wqT = nc.dram_tensor("wqT", [D, D], F32R, kind="ExternalInput").ap()
    wkT = nc.dram_tensor("wkT", [D, D], F32R, kind="ExternalInput").ap()
    wvT = nc.dram_tensor("wvT", [D, D], F32R, kind="ExternalInput").ap()
    bq2d = nc.dram_tensor("bq2d", [128, KT], F32, kind="ExternalInput").ap()
    bk2d = nc.dram_tensor("bk2d", [128, KT], F32, kind="ExternalInput").ap()
    bvrow = nc.dram_tensor("bvrow", [1, D], F32, kind="ExternalInput").ap()
    mask2d = nc.dram_tensor("mask2d", [128, KT], F32, kind="ExternalInput").ap()
    ctxT = nc.dram_tensor("ctxT", [D, S], F32, kind="ExternalOutput").ap()
    # DRAM bounce buffers: rowsums out, reciprocals back (per head, flat 1024)
    rsums = nc.dram_tensor("rsums", [H, NT, 512], F32).ap()
    recips = nc.dram_tensor("recips", [H, NT, 512], F32).ap()

    hT_r = hT.rearrange("(kt p) s -> p kt s", p=128)
    wqT_r = wqT.rearrange("(kt p) o -> p kt o", p=128)
    wkT_r = wkT.rearrange("(kt p) o -> p kt o", p=128)
    wvT_r = wvT.rearrange("(kt p) o -> p kt o", p=128)

    with tile.TileContext(nc) as tc:
        with (
            tc.tile_pool(name="persist", bufs=1) as persist,
            tc.tile_pool(name="wpool", bufs=2) as wpool,
            tc.tile_pool(name="qkpool", bufs=2) as qkpool,
            tc.tile_pool(name="expool", bufs=8) as expool,
            tc.tile_pool(name="outpool", bufs=4) as outpool,
            tc.tile_pool(name="ps", bufs=2, space="PSUM") as ps,
        ):
            # ---- persistent SBUF ----
            # (first hT chunk + head pair 0's weights lead the DMA queues so
            # the first matmul can start within a few microseconds)
            wq0_blk = wpool.tile([128, KT, 128], F32R, tag="wq", name="wq0")
            nc.sync.dma_start(out=wq0_blk, in_=wqT_r[:, :, _ts(0, 128)])
            wk0_blk = wpool.tile([128, KT, 128], F32R, tag="wk", name="wk0")
            nc.sync.dma_start(out=wk0_blk, in_=wkT_r[:, :, _ts(0, 128)])
            hT_sb = persist.tile([128, KT, S], F32R)
            for kt in range(KT):
                for hh in range(2):
                    nc.sync.dma_start(
                        out=hT_sb[:, kt, _ts(hh, 512)], in_=hT_r[:, kt, _ts(hh, 512)]
                    )
            bq_sb = persist.tile([128, KT], F32)
            nc.sync.dma_start(out=bq_sb, in_=bq2d)
            bk_sb = persist.tile([128, KT], F32)
            nc.sync.dma_start(out=bk_sb, in_=bk2d)
            mask_sb = persist.tile([128, KT], F32)
            nc.sync.dma_start(out=mask_sb, in_=mask2d)
            # bv broadcast to all partitions (partition-step-0 DMA from DRAM)
            bv_bc = persist.tile([128, D], F32)
            nc.sync.dma_start(
                out=bv_bc,
                in_=bass.AP(tensor=bvrow.tensor, offset=0, ap=[[0, 128], [1, D]]),
            )
            ones_f = persist.tile([128, H], F32)
            nc.vector.memset(ones_f, 1.0)

            # V in token-major layout, one 65-wide block per head
            # ([64 cols of V_h | ones]); the ones column yields the softmax
            # denominator for free during the PV matmul.
            v_sb = persist.tile([128, KT, H * (DH + 1)], F32R)
            v4 = v_sb.rearrange("p st (h c) -> p st h c", c=DH + 1)
            for st in range(KT):
                nc.vector.tensor_copy(
                    v4[:, st, :, DH : DH + 1],
                    ones_f.rearrange("p (h o) -> p h o", o=1),
                )

            qk_tiles = {}

            def emit_qk(hp, preloaded=None):
                """Q^T/K^T projection for head pair hp, yielded in small pieces
                so the caller can interleave them into attention emission."""
                if preloaded is not None:
                    wq_blk, wk_blk = preloaded
                else:
                    wq_blk = wpool.tile([128, KT, 128], F32R, tag="wq", name=f"wq{hp}")
                    nc.sync.dma_start(out=wq_blk, in_=wqT_r[:, :, _ts(hp, 128)])
                    wk_blk = wpool.tile([128, KT, 128], F32R, tag="wk", name=f"wk{hp}")
                    nc.sync.dma_start(out=wk_blk, in_=wkT_r[:, :, _ts(hp, 128)])
                res = []
                for pi, (blk, bias, tg) in enumerate(
                    ((wq_blk, bq_sb, "qT"), (wk_blk, bk_sb, "kT"))
                ):
                    t = qkpool.tile([128, S], F32R, tag=tg, name=f"{tg}{hp}")
                    for nt in range(NT):
                        p0 = ps.tile(
                            [128, 512], F32, tag="qk", bufs=1, name=f"pq{hp}_{pi}{nt}"
                        )
                        for kt in range(KT):
                            nc.tensor.matmul(
                                p0,
                                blk[:, kt, :],
                                hT_sb[:, kt, _ts(nt, 512)],
                                start=(kt == 0),
                                stop=(kt == KT - 1),
                            )
                            if kt % 2 == 1:
                                yield
                        nc.vector.tensor_scalar_add(
                            t[:, _ts(nt, 512)], p0, bias[:, hp : hp + 1]
                        )
                        yield
                    res.append(t)
                qk_tiles[hp] = res

            # head pair 0's projections up-front: weight DMAs + first matmuls
            # lead, then the remaining hT chunks, then the rest.
            for _ in emit_qk(0, preloaded=(wq0_blk, wk0_blk)):
                pass

            # ---- V projection: V[s, o] = sum_d H^T[d, s] Wv^T[d, o] + bv[o] ----
            wvT_sb = persist.tile([128, KT, D], F32R)
            for kt in range(KT):
                for hh in range(2):
                    nc.sync.dma_start(
                        out=wvT_sb[:, kt, _ts(hh, 512)], in_=wvT_r[:, kt, _ts(hh, 512)]
                    )
            for st in range(KT):
                for nt in range(NT):
                    ps_v = ps.tile([128, 512], F32, tag="pv", bufs=3, name=f"psv{st}_{nt}")
                    for kt in range(KT):
                        nc.tensor.matmul(
                            ps_v,
                            hT_sb[:, kt, _ts(st, 128)],
                            wvT_sb[:, kt, _ts(nt, 512)],
                            start=(kt == 0),
                            stop=(kt == KT - 1),
                        )
                    # scatter into v_sb with the bias added on the way
                    nc.vector.tensor_tensor(
                        out=v4[:, st, 8 * nt : 8 * nt + 8, 0:DH],
                        in0=ps_v.rearrange("p (h c) -> p h c", c=DH),
                        in1=bv_bc[:, _ts(nt, 512)].rearrange("p (h c) -> p h c", c=DH),
                        op=mybir.AluOpType.add,
                    )

            # ---- attention, one-deep software pipeline over (pair, head, mt):
            # PV matmuls for unit n are emitted after unit n+1's scores+exp so
            # they never head-of-line-block the PE queue while exp(n) runs.
            pv_tiles = {}

            def emit_pv(hp, hl, mt, ex):
                h = 2 * hp + hl
                if mt == 0:
                    pv_tiles[h] = [
                        ps.tile(
                            [DH + 1, 512], F32, tag="pv", bufs=3, name=f"pspv{h}_{i}"
                        )
                        for i in range(NT)
                    ]
                for nt in range(NT):
                    nc.tensor.matmul(
                        pv_tiles[h][nt],
                        v_sb[:, mt, h * (DH + 1) : (h + 1) * (DH + 1)],
                        ex[:, _ts(nt, 512)],
                        start=(mt == 0),
                        stop=(mt == KT - 1),
                    )
                if mt == KT - 1:
                    emit_norm(h)

            def emit_norm(h):
                # rowsum row -> DRAM, batched reciprocal on [128, 8] (all
                # lanes), back to DRAM, partition-broadcast loads, final mul.
                ps_pv = pv_tiles[h]
                # copy ctx + rowsum out of PSUM first so the banks free quickly
                rs_sb = outpool.tile([DH + 1, NT, 512], F32, tag="rs", bufs=2, name=f"rs{h}")
                cs_sb = outpool.tile([DH, NT, 512], F32, tag="cs", bufs=2, name=f"cs{h}")
                for nt in range(NT):
                    nc.vector.tensor_copy(
                        rs_sb[DH : DH + 1, nt, :], ps_pv[nt][DH : DH + 1, :]
                    )
                    nc.vector.tensor_copy(cs_sb[:, nt, :], ps_pv[nt][0:DH, :])
                    nc.sync.dma_start(out=rsums[h, nt, :], in_=rs_sb[DH : DH + 1, nt, :])
                rc_sb = outpool.tile([128, KT], F32, tag="rc", bufs=2, name=f"rc{h}")
                nc.sync.dma_start(
                    out=rc_sb,
                    in_=bass.AP(tensor=rsums.tensor, offset=h * S, ap=[[KT, 128], [1, KT]]),
                )
                nc.vector.reciprocal(rc_sb, rc_sb)
                nc.sync.dma_start(
                    out=bass.AP(
                        tensor=recips.tensor, offset=h * S, ap=[[KT, 128], [1, KT]]
                    ),
                    in_=rc_sb,
                )
                for nt in range(NT):
                    bc_t = outpool.tile([DH, 512], F32, tag="bc", name=f"bc{h}_{nt}")
                    nc.sync.dma_start(
                        out=bc_t,
                        in_=bass.AP(
                            tensor=recips.tensor,
                            offset=h * S + nt * 512,
                            ap=[[0, DH], [1, 512]],
                        ),
                    )
                    stage = outpool.tile([DH, 512], F32, tag="stage", name=f"st{h}_{nt}")
                    nc.vector.tensor_mul(stage, cs_sb[:, nt, :], bc_t)
                    nc.sync.dma_start(
                        out=ctxT[h * DH : (h + 1) * DH, _ts(nt, 512)], in_=stage
                    )

            pending_pv = []
            for hp in range(HP):
                qT_t, kT_t = qk_tiles[hp]
                nxt = emit_qk(hp + 1) if hp + 1 < HP else iter(())
                for hl in range(2):
                    h = 2 * hp + hl
                    base = 64 * hl
                    for mt in range(KT):
                        # S^T[kpos, q] for this head
                        ps_s = ps.tile([128, 1024], F32, tag="sc", name=f"pss{h}_{mt}")
                        for nt in range(NT):
                            nc.tensor.matmul(
                                ps_s[:, _ts(nt, 512)],
                                kT_t[base : base + 64, _ts(mt, 128)],
                                qT_t[base : base + 64, _ts(nt, 512)],
                                start=True,
                                stop=True,
                            )
                        # probs_unnorm = exp(S^T/8 + mask[kpos])
                        ex = expool.tile([128, S], F32R, tag="ex", name=f"ex{h}_{mt}")
                        nc.scalar.activation(
                            ex,
                            ps_s,
                            mybir.ActivationFunctionType.Exp,
                            bias=mask_sb[:, mt : mt + 1],
                            scale=0.125,
                        )
                        pending_pv.append((hp, hl, mt, ex))
                        depth = 1 if (hp == HP - 1 and hl == 1) else 2
                        while len(pending_pv) > depth:
                            emit_pv(*pending_pv.pop(0))
                        next(nxt, None)
                        next(nxt, None)
                # flush any remaining pipelined projection work
                for _ in nxt:
                    pass
            for args in pending_pv:
                emit_pv(*args)
    nc.compile()
    return nc


_NC_CACHE = None


def _get_nc():
    global _NC_CACHE
    if _NC_CACHE is None:
        _NC_CACHE = build_program()
    return _NC_CACHE


def _prep_inputs(hidden_states, attention_mask, head_mask, Wq, bq, Wk, bk, Wv, bv):
    hidden_states = np.asarray(hidden_states, dtype=np.float32)
    attention_mask = np.asarray(attention_mask, dtype=np.float32)
    head_mask = np.asarray(head_mask, dtype=np.float32)
    Wq = np.asarray(Wq, dtype=np.float32)
    bq = np.asarray(bq, dtype=np.float32)
    Wk = np.asarray(Wk, dtype=np.float32)
    bk = np.asarray(bk, dtype=np.float32)
    Wv = np.asarray(Wv, dtype=np.float32)
    bv = np.asarray(bv, dtype=np.float32)

    # fold head_mask into Wv/bv (probs*hm @ V == probs @ (hm*V))
    hm = head_mask.reshape(H)
    hscale = np.repeat(hm, DH).astype(np.float32)
    wqT = np.ascontiguousarray(Wq.T)
    wkT = np.ascontiguousarray(Wk.T)
    wvT = np.ascontiguousarray((Wv * hscale[:, None]).T)
    bq2d = np.ascontiguousarray(bq.reshape(KT, 128).T)
    bk2d = np.ascontiguousarray(bk.reshape(KT, 128).T)
    bvrow = (bv * hscale).reshape(1, D)

    mask = np.broadcast_to(
        attention_mask.reshape(attention_mask.shape[0], -1)[:, -S:], (N_CORES, S)
    )

    in_maps = []
    for b in range(N_CORES):
        in_maps.append(
            {
                "hT": np.ascontiguousarray(hidden_states[b].T),
                "wqT": wqT,
                "wkT": wkT,
                "wvT": wvT,
                "bq2d": bq2d,
                "bk2d": bk2d,
                "bvrow": bvrow,
                "mask2d": np.ascontiguousarray(mask[b].reshape(KT, 128).T),
            }
        )
    return in_maps


def _install_trace_shim():
    """antenv.axon_hooks is absent in this image; provide it so trace=True works."""
    import types

    if "antenv.axon_hooks" in sys.modules:
        return
    mod = types.ModuleType("antenv.axon_hooks")
    mod._hook = None

    def _set(h):
        mod._hook = h

    def _get():
        return mod._hook

    mod.set_axon_ntff_profile_hook = _set
    mod.get_axon_ntff_profile_hook = _get
    sys.modules["antenv.axon_hooks"] = mod
    try:
        from trn_agent_boot.trn_boot import _ntff_profile_via_ctypes

        _set(_ntff_profile_via_ctypes("/opt/axon/libaxon_pjrt.so"))
    except Exception:
        pass


def _kernel_impl(trace=False, **inputs):
    nc = _get_nc()
    in_maps = _prep_inputs(**inputs)
    kwargs = {}
    if trace:
        _install_trace_shim()
        kwargs["trace"] = True
        kwargs["trace_cores"] = list(range(N_CORES))
    res = run_bass_kernel_spmd(nc, in_maps, core_ids=list(range(N_CORES)), **kwargs)
    out = np.empty((N_CORES, S, D), dtype=np.float32)
    for b in range(N_CORES):
        out[b] = res.results[b]["ctxT"].T
    return out, res


def kernel(**inputs) -> np.ndarray:
    return _kernel_impl(trace=False, **inputs)[0]



# revision 4
# speedup vs baseline: 1.0652x; 1.0652x over previous
"""MoEBertSelfAttention on 8 Trainium2 NeuronCores.

Strategy: data-parallel over batch (B=8 -> one batch element per core).
Each core computes its element's full self-attention.

v2 design (PE-array tiling to cut effective matmul rows):
  - all matmul operands in bf16 (fp32 PSUM accumulation),
  - scores: the two heads of a head pair run as two CONCURRENT row-tiled
    matmuls (each K=64, tile_position (0,0)/(64,0)) writing the two halves
    of one [128, 1024] PSUM tile -> 2x score throughput, one 1024-wide
    exp() per (pair, kpos-chunk, qpos-half),
  - PV: the two heads run as two CONCURRENT col-tiled matmuls (each M=64,
    tile_position (0,0)/(0,64)) accumulating into one [128, 512] PSUM tile
    (partitions 0-63 = head h dims, 64-127 = head h' dims) -> 2x PV
    throughput,
  - softmax denominators via four CONCURRENT col-tiled ones-matmuls (M=1
    at partitions 0/32/64/96) accumulating over kpos chunks,
  - normalization happens on the host: kernel returns unnormalized ctx^T
    plus per-head denominators; host divides (exact same softmax ratio
    since numerator and denominator use the same rounded exp values),
  - head_mask folded into Wv/bv on the host (exact); attention mask is a
    per-partition bias on the exp() activation.

Pipelining: per unit (pair, kpos-chunk mt, qpos-half nt) the PE runs the
paired score matmuls; PV+den matmuls of the previous unit and projection
work for the next pair / V blocks fill the PE while the ACT engine runs
exp(), so both engines stay busy.
"""

import sys

if "/opt/trn_rl_repo" not in sys.path:
    sys.path.insert(0, "/opt/trn_rl_repo")

import numpy as np

import concourse.bacc as bacc
import concourse.bass as bass
import concourse.tile as tile
from concourse import mybir
from concourse.bass_utils import run_bass_kernel_spmd

S = 1024  # sequence length
D = 1024  # hidden size
H = 16  # heads
DH = 64  # head size
KT = D // 128  # 128-row tiles along a feature dim
NT = S // 512  # 512-col tiles along the sequence
HP = H // 2  # head pairs
N_CORES = 8

F32 = mybir.dt.float32
BF16 = mybir.dt.bfloat16


def _ts(i, n):
    return slice(i * n, (i + 1) * n)


def build_program():
    nc = bacc.Bacc("TRN2", target_bir_lowering=False, debug=False, num_devices=N_CORES)

    hT = nc.dram_tensor("hT", [D, S], BF16, kind="ExternalInput").ap()
    wqT = nc.dram_tensor("wqT", [D, D], BF16, kind="ExternalInput").ap()
    wkT = nc.dram_tensor("wkT", [D, D], BF16, kind="ExternalInput").ap()
    wvT = nc.dram_tensor("wvT", [D, D], BF16, kind="ExternalInput").ap()
    bq2d = nc.dram_tensor("bq2d", [128, KT], F32, kind="ExternalInput").ap()
    bk2d = nc.dram_tensor("bk2d", [128, KT], F32, kind="ExternalInput").ap()
    bvrow = nc.dram_tensor("bvrow", [1, D], F32, kind="ExternalInput").ap()
    mask2d = nc.dram_tensor("mask2d", [128, KT], F32, kind="ExternalInput").ap()
    ctxT = nc.dram_tensor("ctxT", [D, S], F32, kind="ExternalOutput").ap()
    dens = nc.dram_tensor("dens", [HP, 128, 512], F32, kind="ExternalOutput").ap()

    hT_r = hT.rearrange("(kt p) s -> p kt s", p=128)
    wqT_r = wqT.rearrange("(kt p) o -> p kt o", p=128)
    wkT_r = wkT.rearrange("(kt p) o -> p kt o", p=128)
    wvT_r = wvT.rearrange("(kt p) o -> p kt o", p=128)

    with tile.TileContext(nc) as tc:
        with (
            tc.tile_pool(name="persist", bufs=1) as persist,
            tc.tile_pool(name="wpool", bufs=2) as wpool,
            tc.tile_pool(name="qkpool", bufs=2) as qkpool,
            tc.tile_pool(name="expool", bufs=6) as expool,
            tc.tile_pool(name="outpool", bufs=2) as outpool,
            tc.tile_pool(name="ps", bufs=1, space="PSUM") as ps,
        ):
            # ---- persistent SBUF ----
            # head pair 0's weights + first hT chunks lead the DMA queues
            wq0_blk = wpool.tile([128, KT, 128], BF16, tag="wq", name="wq0")
            nc.sync.dma_start(out=wq0_blk, in_=wqT_r[:, :, _ts(0, 128)])
            wk0_blk = wpool.tile([128, KT, 128], BF16, tag="wk", name="wk0")
            nc.sync.dma_start(out=wk0_blk, in_=wkT_r[:, :, _ts(0, 128)])
            hT_sb = persist.tile([128, KT, S], BF16)
            for kt in range(KT):
                for hh in range(2):
                    nc.sync.dma_start(
                        out=hT_sb[:, kt, _ts(hh, 512)], in_=hT_r[:, kt, _ts(hh, 512)]
                    )
            bq_sb = persist.tile([128, KT], F32)
            nc.sync.dma_start(out=bq_sb, in_=bq2d)
            bk_sb = persist.tile([128, KT], F32)
            nc.sync.dma_start(out=bk_sb, in_=bk2d)
            mask_sb = persist.tile([128, KT], F32)
            nc.sync.dma_start(out=mask_sb, in_=mask2d)
            # bv broadcast to all partitions (partition-step-0 DMA from DRAM)
            bv_bc = persist.tile([128, D], F32)
            nc.sync.dma_start(
                out=bv_bc,
                in_=bass.AP(tensor=bvrow.tensor, offset=0, ap=[[0, 128], [1, D]]),
            )
            ones_sb = persist.tile([128, 1], BF16)
            nc.vector.memset(ones_sb, 1.0)

            wvT_sb = persist.tile([128, KT, D], BF16)
            for kt in range(KT):
                for hh in range(2):
                    nc.sync.dma_start(
                        out=wvT_sb[:, kt, _ts(hh, 512)], in_=wvT_r[:, kt, _ts(hh, 512)]
                    )

            # V in token-major layout: v4[:, st, h, 0:64]
            v_sb = persist.tile([128, KT, H * DH], BF16)
            v4 = v_sb.rearrange("p st (h c) -> p st h c", c=DH)

            qk_tiles = {}

            def emit_qk(hp, preloaded=None):
                """Q^T/K^T projection for head pair hp, yielded one
                (tensor, nt) psum-group at a time (group-atomic: the shared
                proj psum slot must not interleave two accumulation groups)."""
                if preloaded is not None:
                    wq_blk, wk_blk = preloaded
                else:
                    wq_blk = wpool.tile([128, KT, 128], BF16, tag="wq", name=f"wq{hp}")
                    nc.sync.dma_start(out=wq_blk, in_=wqT_r[:, :, _ts(hp, 128)])
                    wk_blk = wpool.tile([128, KT, 128], BF16, tag="wk", name=f"wk{hp}")
                    nc.sync.dma_start(out=wk_blk, in_=wkT_r[:, :, _ts(hp, 128)])
                res = []
                for pi, (blk, bias, tg) in enumerate(
                    ((wq_blk, bq_sb, "qT"), (wk_blk, bk_sb, "kT"))
                ):
                    t = qkpool.tile([128, S], BF16, tag=tg, name=f"{tg}{hp}")
                    for nt in range(NT):
                        p0 = ps.tile(
                            [128, 512], F32, tag="proj", bufs=1, name=f"pq{hp}_{pi}{nt}"
                        )
                        for kt in range(KT):
                            nc.tensor.matmul(
                                p0,
                                blk[:, kt, :],
                                hT_sb[:, kt, _ts(nt, 512)],
                                start=(kt == 0),
                                stop=(kt == KT - 1),
                            )
                        nc.vector.tensor_scalar_add(
                            t[:, _ts(nt, 512)], p0, bias[:, hp : hp + 1]
                        )
                        yield
                    res.append(t)
                qk_tiles[hp] = res

            def emit_v():
                """V projection, one (st, nt) psum-group per yield.
                V[s, o] = sum_d H^T[d, s] Wv^T[d, o] + bv[o]."""
                for nt in range(NT):
                    for st in range(KT):
                        ps_v = ps.tile(
                            [128, 512], F32, tag="proj", bufs=1, name=f"psv{st}_{nt}"
                        )
                        for kt in range(KT):
                            nc.tensor.matmul(
                                ps_v,
                                hT_sb[:, kt, _ts(st, 128)],
                                wvT_sb[:, kt, _ts(nt, 512)],
                                start=(kt == 0),
                                stop=(kt == KT - 1),
                            )
                        nc.vector.tensor_tensor(
                            out=v4[:, st, 8 * nt : 8 * nt + 8, :],
                            in0=ps_v.rearrange("p (h c) -> p h c", c=DH),
                            in1=bv_bc[:, _ts(nt, 512)].rearrange(
                                "p (h c) -> p h c", c=DH
                            ),
                            op=mybir.AluOpType.add,
                        )
                        yield

            # head pair 0's projections up-front
            for _ in emit_qk(0, preloaded=(wq0_blk, wk0_blk)):
                pass

            # ---- attention ----
            pv_ps = {}
            den_ps = {}
            ex_keep = {}

            def emit_pv(hp, mt, nt, ex):
                """Paired PV + denominator matmuls for unit (hp, mt, nt)."""
                if mt == 0 and nt == 0:
                    pv_ps[hp] = [
                        ps.tile([128, 512], F32, tag="pv", bufs=2, name=f"pspv{hp}_{i}")
                        for i in range(NT)
                    ]
                    den_ps[hp] = ps.tile(
                        [128, 512], F32, tag="den", bufs=1, name=f"psden{hp}"
                    )
                pvt = pv_ps[hp]
                start, stop = mt == 0, mt == KT - 1
                # ctx_h^T -> partitions 0:64 ; ctx_h'^T -> partitions 64:128
                nc.tensor.matmul(
                    pvt[nt][0:64, :],
                    v4[:, mt, 2 * hp, :],
                    ex[:, 0:512],
                    start=start,
                    stop=stop,
                    tile_position=(0, 0),
                )
                nc.tensor.matmul(
                    pvt[nt][64:128, :],
                    v4[:, mt, 2 * hp + 1, :],
                    ex[:, 512:1024],
                    start=start,
                    stop=stop,
                    tile_position=(0, 64),
                )
                if nt == 0:
                    ex_keep[(hp, mt)] = ex
                    return
                # denominators: four concurrent single-row ones-matmuls in
                # distinct 32-col groups: (h,nt0)->0 (h',nt0)->32
                # (h,nt1)->64 (h',nt1)->96
                ex0 = ex_keep.pop((hp, mt))
                dent = den_ps[hp]
                for pbase, src in (
                    (0, ex0[:, 0:512]),
                    (32, ex0[:, 512:1024]),
                    (64, ex[:, 0:512]),
                    (96, ex[:, 512:1024]),
                ):
                    nc.tensor.matmul(
                        dent[pbase : pbase + 1, :],
                        ones_sb,
                        src,
                        start=start,
                        stop=stop,
                        tile_position=(0, pbase),
                    )
                if stop:
                    emit_out(hp)

            def emit_out(hp):
                # drain PSUM -> SBUF -> DRAM (DMA cannot read PSUM)
                den_sb = outpool.tile([128, 512], F32, tag="den", name=f"den{hp}")
                nc.vector.tensor_copy(den_sb, den_ps[hp])
                nc.sync.dma_start(out=dens[hp], in_=den_sb)
                for nt in range(NT):
                    c_sb = outpool.tile([128, 512], F32, tag="ctx", name=f"c{hp}_{nt}")
                    nc.vector.tensor_copy(c_sb, pv_ps[hp][nt])
                    nc.sync.dma_start(
                        out=ctxT[_ts(hp, 128), _ts(nt, 512)], in_=c_sb
                    )

            # filler: next-pair QK projections, then V blocks (group-atomic)
            pending = []
            fillers = []

            def pull_filler(n):
                cnt = 0
                while fillers and cnt < n:
                    if next(fillers[0], "END") == "END":
                        fillers.pop(0)
                    else:
                        cnt += 1

            v_gen = emit_v()
            for hp in range(HP):
                qT_t, kT_t = qk_tiles[hp]
                if hp + 1 < HP:
                    fillers.append(emit_qk(hp + 1))
                for mt in range(KT):
                    for nt in range(NT):
                        # paired scores: S_h^T -> cols 0:512, S_h'^T -> 512:1024
                        ps_s = ps.tile(
                            [128, 1024], F32, tag="sc", bufs=2, name=f"pss{hp}_{mt}{nt}"
                        )
                        nc.tensor.matmul(
                            ps_s[:, 0:512],
                            kT_t[0:64, _ts(mt, 128)],
                            qT_t[0:64, _ts(nt, 512)],
                            start=True,
                            stop=True,
                            tile_position=(0, 0),
                        )
                        nc.tensor.matmul(
                            ps_s[:, 512:1024],
                            kT_t[64:128, _ts(mt, 128)],
                            qT_t[64:128, _ts(nt, 512)],
                            start=True,
                            stop=True,
                            tile_position=(64, 0),
                        )
                        # probs_unnorm = exp(S^T/8 + mask[kpos]) in bf16
                        ex = expool.tile(
                            [128, 1024], BF16, tag="ex", name=f"ex{hp}_{mt}{nt}"
                        )
                        nc.scalar.activation(
                            ex,
                            ps_s,
                            mybir.ActivationFunctionType.Exp,
                            bias=mask_sb[:, mt : mt + 1],
                            scale=0.125,
                        )
                        pending.append((hp, mt, nt, ex))
                        depth = 1 if (hp == HP - 1) else 2
                        while len(pending) > depth:
                            emit_pv(*pending.pop(0))
                    # one V-projection block per kpos chunk keeps v4[:, mt+1]
                    # ready ahead of PV(hp=0, mt+1); later pairs drain the rest
                    if next(v_gen, "END") != "END":
                        pass
                    pull_filler(1)
            for args in pending:
                emit_pv(*args)
    nc.compile()
    return nc


_NC_CACHE = None


def _get_nc():
    global _NC_CACHE
    if _NC_CACHE is None:
        _NC_CACHE = build_program()
    return _NC_CACHE


def _prep_inputs(hidden_states, attention_mask, head_mask, Wq, bq, Wk, bk, Wv, bv):
    import ml_dtypes

    bf16 = ml_dtypes.bfloat16
    hidden_states = np.asarray(hidden_states, dtype=np.float32)
    attention_mask = np.asarray(attention_mask, dtype=np.float32)
    head_mask = np.asarray(head_mask, dtype=np.float32)
    Wq = np.asarray(Wq, dtype=np.float32)
    bq = np.asarray(bq, dtype=np.float32)
    Wk = np.asarray(Wk, dtype=np.float32)
    bk = np.asarray(bk, dtype=np.float32)
    Wv = np.asarray(Wv, dtype=np.float32)
    bv = np.asarray(bv, dtype=np.float32)

    # fold head_mask into Wv/bv (probs*hm @ V == probs @ (hm*V)); the
    # denominator is computed from raw exp values so it stays unscaled.
    hm = head_mask.reshape(H)
    hscale = np.repeat(hm, DH).astype(np.float32)
    wqT = np.ascontiguousarray(Wq.T.astype(bf16))
    wkT = np.ascontiguousarray(Wk.T.astype(bf16))
    wvT = np.ascontiguousarray((Wv * hscale[:, None]).T.astype(bf16))
    bq2d = np.ascontiguousarray(bq.reshape(KT, 128).T)
    bk2d = np.ascontiguousarray(bk.reshape(KT, 128).T)
    bvrow = (bv * hscale).reshape(1, D)

    mask = np.broadcast_to(
        attention_mask.reshape(attention_mask.shape[0], -1)[:, -S:], (N_CORES, S)
    )

    in_maps = []
    for b in range(N_CORES):
        in_maps.append(
            {
                "hT": np.ascontiguousarray(hidden_states[b].T.astype(bf16)),
                "wqT": wqT,
                "wkT": wkT,
                "wvT": wvT,
                "bq2d": bq2d,
                "bk2d": bk2d,
                "bvrow": bvrow,
                "mask2d": np.ascontiguousarray(mask[b].reshape(KT, 128).T),
            }
        )
    return in_maps


def _install_trace_shim():
    """antenv.axon_hooks is absent in this image; provide it so trace=True works."""
    import types

    if "antenv.axon_hooks" in sys.modules:
        return
    mod = types.ModuleType("antenv.axon_hooks")
    mod._hook = None

    def _set(h):
        mod._hook = h

    def _get():
        return mod._hook

    mod.set_axon_ntff_profile_hook = _set
    mod.get_axon_ntff_profile_hook = _get
    sys.modules["antenv.axon_hooks"] = mod
    try:
        from trn_agent_boot.trn_boot import _ntff_profile_via_ctypes

        _set(_ntff_profile_via_ctypes("/opt/axon/libaxon_pjrt.so"))
    except Exception:
        pass


def _kernel_impl(trace=False, **inputs):
    nc = _get_nc()
    in_maps = _prep_inputs(**inputs)
    kwargs = {}
    if trace:
        _install_trace_shim()
        kwargs["trace"] = True
        kwargs["trace_cores"] = list(range(N_CORES))
    res = run_bass_kernel_spmd(nc, in_maps, core_ids=list(range(N_CORES)), **kwargs)
    out = np.empty((N_CORES, S, D), dtype=np.float32)
    den_full = np.empty((H, S), dtype=np.float32)
    for b in range(N_CORES):
        ctxu = np.asarray(res.results[b]["ctxT"], dtype=np.float32)
        denf = np.asarray(res.results[b]["dens"], dtype=np.float32)
        for hp in range(HP):
            den_full[2 * hp, 0:512] = denf[hp, 0]
            den_full[2 * hp + 1, 0:512] = denf[hp, 32]
            den_full[2 * hp, 512:1024] = denf[hp, 64]
            den_full[2 * hp + 1, 512:1024] = denf[hp, 96]
        out[b] = (ctxu / np.repeat(den_full, DH, axis=0)).T
    return out, res


def kernel(**inputs) -> np.ndarray:
    return _kernel_impl(trace=False, **inputs)[0]


# revision 9
# speedup vs baseline: 1.2054x; 1.1316x over previous
"""MoEBertSelfAttention on 8 Trainium2 NeuronCores.

Strategy: data-parallel over batch (B=8 -> one batch element per core).
Each core computes its element's full self-attention.

v3 design:
  - on-device dataflow fully transposed (no on-chip transposes): host passes
    H^T / W^T; scores are computed as S^T (key position on partitions) so
    the additive attention mask is a per-partition bias on the exp()
    activation,
  - all matmul operands bf16 (fp32 PSUM), except the Q/K projections which
    run in fp8(e4m3) with perf_mode=DoubleRow: two 128-deep contraction
    chunks packed per matmul -> ~1.5x projection throughput. The fp8
    rounding only perturbs attention logits (~4% of their unit-scale std);
    the softmax ratio cancels most of the downstream effect,
  - V projection stays bf16 (its rounding lands directly in the output),
  - the softmax denominator rides as an extra all-ones bf16 column of V in
    the PV matmul ([64 cols of V_h | ones] per head),
  - normalization happens on the host: the kernel returns unnormalized
    ctx^T plus the denominator rows; the host divides. Numerator and
    denominator use the same rounded exp values, so the softmax ratio is
    exact up to fp32 accumulation,
  - head_mask folded into Wv/bv on the host (exact).

Pipelining: per (head, kpos-chunk) unit the PE runs two 512-wide score
matmuls; the PV matmuls of older units plus projection psum-groups for the
next head pair / V blocks fill the PE while ACT runs exp(), keeping the PE
queue dense so it stays at the 2.4 GHz p-state.
"""

import sys

if "/opt/trn_rl_repo" not in sys.path:
    sys.path.insert(0, "/opt/trn_rl_repo")

import numpy as np

import concourse.bacc as bacc
import concourse.bass as bass
import concourse.tile as tile
from concourse import mybir
from concourse.bass_utils import run_bass_kernel_spmd

S = 1024  # sequence length
D = 1024  # hidden size
H = 16  # heads
DH = 64  # head size
KT = D // 128  # 128-row tiles along a feature dim
NT = S // 512  # 512-col tiles along the sequence
HP = H // 2  # head pairs
N_CORES = 8

F32 = mybir.dt.float32
BF16 = mybir.dt.bfloat16
FP8 = mybir.dt.float8e4

QK_FP8 = False  # fp8 Q/K projections: measured rel err 4.8e-2 > 2e-2 gate


def _ts(i, n):
    return slice(i * n, (i + 1) * n)


def build_program():
    nc = bacc.Bacc("TRN2", target_bir_lowering=False, debug=False, num_devices=N_CORES)

    qk_dt = BF16
    hTb = nc.dram_tensor("hTb", [D, S], BF16, kind="ExternalInput").ap()
    wqT = nc.dram_tensor("wqT", [D, D], qk_dt, kind="ExternalInput").ap()
    wkT = nc.dram_tensor("wkT", [D, D], qk_dt, kind="ExternalInput").ap()
    wvT = nc.dram_tensor("wvT", [D, D], BF16, kind="ExternalInput").ap()
    bq2d = nc.dram_tensor("bq2d", [128, KT], F32, kind="ExternalInput").ap()
    bk2d = nc.dram_tensor("bk2d", [128, KT], F32, kind="ExternalInput").ap()
    bvrow = nc.dram_tensor("bvrow", [1, D], F32, kind="ExternalInput").ap()
    mask2d = nc.dram_tensor("mask2d", [128, KT], F32, kind="ExternalInput").ap()
    ctxT = nc.dram_tensor("ctxT", [D, S], F32, kind="ExternalOutput").ap()
    dens = nc.dram_tensor("dens", [H, NT, 512], F32, kind="ExternalOutput").ap()

    hTb_r = hTb.rearrange("(kt p) s -> p kt s", p=128)
    wqT_r = wqT.rearrange("(kt p) o -> p kt o", p=128)
    wkT_r = wkT.rearrange("(kt p) o -> p kt o", p=128)
    wvT_r = wvT.rearrange("(kt p) o -> p kt o", p=128)

    with tile.TileContext(nc) as tc:
        with (
            tc.tile_pool(name="persist", bufs=1) as persist,
            tc.tile_pool(name="wpool", bufs=2) as wpool,
            tc.tile_pool(name="qkpool", bufs=2) as qkpool,
            tc.tile_pool(name="expool", bufs=6) as expool,
            tc.tile_pool(name="outpool", bufs=3) as outpool,
            tc.tile_pool(name="ps", bufs=1, space="PSUM") as ps,
        ):
            # ---- persistent SBUF ----
            # head pair 0's weights + the QK copy of hT lead the DMA queues
            wq0_blk = wpool.tile([128, KT, 128], qk_dt, tag="wq", name="wq0")
            nc.sync.dma_start(out=wq0_blk, in_=wqT_r[:, :, _ts(0, 128)])
            wk0_blk = wpool.tile([128, KT, 128], qk_dt, tag="wk", name="wk0")
            nc.sync.dma_start(out=wk0_blk, in_=wkT_r[:, :, _ts(0, 128)])
            bq_sb = persist.tile([128, KT], F32)
            nc.sync.dma_start(out=bq_sb, in_=bq2d)
            bk_sb = persist.tile([128, KT], F32)
            nc.sync.dma_start(out=bk_sb, in_=bk2d)
            mask_sb = persist.tile([128, KT], F32)
            nc.sync.dma_start(out=mask_sb, in_=mask2d)
            # bv broadcast to all partitions (partition-step-0 DMA from DRAM)
            bv_bc = persist.tile([128, D], F32)
            nc.sync.dma_start(
                out=bv_bc,
                in_=bass.AP(tensor=bvrow.tensor, offset=0, ap=[[0, 128], [1, D]]),
            )
            hb_sb = persist.tile([128, KT, S], BF16)
            for hh in range(2):
                for kt in range(KT):
                    nc.sync.dma_start(
                        out=hb_sb[:, kt, _ts(hh, 512)], in_=hTb_r[:, kt, _ts(hh, 512)]
                    )
            h8_sb = hb_sb
            wvT_sb = persist.tile([128, KT, D], BF16)
            for kt in range(KT):
                for hh in range(2):
                    nc.sync.dma_start(
                        out=wvT_sb[:, kt, _ts(hh, 512)], in_=wvT_r[:, kt, _ts(hh, 512)]
                    )

            ones_f = persist.tile([128, H], BF16)
            nc.vector.memset(ones_f, 1.0)

            # V in token-major layout, one 65-wide block per head
            # ([64 cols of V_h | ones]); the ones column yields the softmax
            # denominator for free during the PV matmul.
            v_sb = persist.tile([128, KT, H * (DH + 1)], BF16)
            v4 = v_sb.rearrange("p st (h c) -> p st h c", c=DH + 1)
            for st in range(KT):
                nc.vector.tensor_copy(
                    v4[:, st, :, DH : DH + 1],
                    ones_f.rearrange("p (h o) -> p h o", o=1),
                )

            qk_tiles = {}

            def emit_qk(hp, preloaded=None):
                """Q^T/K^T projection for head pair hp, yielded one
                (tensor, nt) psum-group at a time (group-atomic: the shared
                proj psum slot must not interleave two accumulation groups)."""
                if preloaded is not None:
                    wq_blk, wk_blk = preloaded
                else:
                    wq_blk = wpool.tile([128, KT, 128], qk_dt, tag="wq", name=f"wq{hp}")
                    nc.sync.dma_start(out=wq_blk, in_=wqT_r[:, :, _ts(hp, 128)])
                    wk_blk = wpool.tile([128, KT, 128], qk_dt, tag="wk", name=f"wk{hp}")
                    nc.sync.dma_start(out=wk_blk, in_=wkT_r[:, :, _ts(hp, 128)])
                res = []
                for pi, (blk, bias, tg) in enumerate(
                    ((wq_blk, bq_sb, "qT"), (wk_blk, bk_sb, "kT"))
                ):
                    t = qkpool.tile([128, S], BF16, tag=tg, name=f"{tg}{hp}")
                    for nt in range(NT):
                        p0 = ps.tile(
                            [128, 512], F32, tag="proj", bufs=1, name=f"pq{hp}_{pi}{nt}"
                        )
                        if QK_FP8:
                            # DoubleRow: two 128-deep contraction chunks per mm
                            for kk in range(KT // 2):
                                nc.tensor.matmul(
                                    p0,
                                    blk[:, 2 * kk : 2 * kk + 2, :],
                                    h8_sb[:, 2 * kk : 2 * kk + 2, _ts(nt, 512)],
                                    start=(kk == 0),
                                    stop=(kk == KT // 2 - 1),
                                    perf_mode=mybir.MatmulPerfMode.DoubleRow,
                                )
                        else:
                            for kt in range(KT):
                                nc.tensor.matmul(
                                    p0,
                                    blk[:, kt, :],
                                    h8_sb[:, kt, _ts(nt, 512)],
                                    start=(kt == 0),
                                    stop=(kt == KT - 1),
                                )
                        nc.vector.tensor_scalar_add(
                            t[:, _ts(nt, 512)], p0, bias[:, hp : hp + 1]
                        )
                        yield
                    res.append(t)
                qk_tiles[hp] = res

            def emit_v():
                """V projection, one (st, nt) psum-group per yield.
                V[s, o] = sum_d H^T[d, s] Wv^T[d, o] + bv[o]."""
                for nt in range(NT):
                    for st in range(KT):
                        ps_v = ps.tile(
                            [128, 512], F32, tag="proj", bufs=1, name=f"psv{st}_{nt}"
                        )
                        for kt in range(KT):
                            nc.tensor.matmul(
                                ps_v,
                                hb_sb[:, kt, _ts(st, 128)],
                                wvT_sb[:, kt, _ts(nt, 512)],
                                start=(kt == 0),
                                stop=(kt == KT - 1),
                            )
                        nc.vector.tensor_tensor(
                            out=v4[:, st, 8 * nt : 8 * nt + 8, 0:DH],
                            in0=ps_v.rearrange("p (h c) -> p h c", c=DH),
                            in1=bv_bc[:, _ts(nt, 512)].rearrange(
                                "p (h c) -> p h c", c=DH
                            ),
                            op=mybir.AluOpType.add,
                        )
                        yield

            # head pair 0's projections up-front
            for _ in emit_qk(0, preloaded=(wq0_blk, wk0_blk)):
                pass

            # ---- attention ----
            pv_tiles = {}

            def emit_pv(h, mt, ex):
                if mt == 0:
                    pv_tiles[h] = [
                        ps.tile(
                            [DH + 1, 512], F32, tag="pv", bufs=3, name=f"pspv{h}_{i}"
                        )
                        for i in range(NT)
                    ]
                for nt in range(NT):
                    nc.tensor.matmul(
                        pv_tiles[h][nt],
                        v_sb[:, mt, h * (DH + 1) : (h + 1) * (DH + 1)],
                        ex[:, _ts(nt, 512)],
                        start=(mt == 0),
                        stop=(mt == KT - 1),
                    )
                if mt == KT - 1:
                    emit_out(h)

            def emit_out(h):
                # drain PSUM -> SBUF -> DRAM (ctx rows 0:64, denominator row 64)
                for nt in range(NT):
                    c_sb = outpool.tile(
                        [DH + 1, 512], F32, tag="ctx", name=f"c{h}_{nt}"
                    )
                    nc.vector.tensor_copy(c_sb, pv_tiles[h][nt])
                    nc.sync.dma_start(
                        out=ctxT[_ts(h, DH), _ts(nt, 512)], in_=c_sb[0:DH, :]
                    )
                    nc.sync.dma_start(out=dens[h, nt, :], in_=c_sb[DH : DH + 1, :])

            # filler: next-pair QK projections, then V blocks (group-atomic)
            pending = []
            fillers = []

            def pull_filler(n):
                cnt = 0
                while fillers and cnt < n:
                    if next(fillers[0], "END") == "END":
                        fillers.pop(0)
                    else:
                        cnt += 1

            v_gen = emit_v()
            unit = 0
            qk_owed = 0.0
            for hp in range(HP):
                qT_t, kT_t = qk_tiles[hp]
                if hp + 1 < HP:
                    fillers.append(emit_qk(hp + 1))
                for hl in range(2):
                    h = 2 * hp + hl
                    base = 64 * hl
                    for mt in range(KT):
                        # S^T[kpos, q] for this head
                        ps_s = ps.tile(
                            [128, 1024], F32, tag="sc", bufs=2, name=f"pss{h}_{mt}"
                        )
                        for nt in range(NT):
                            nc.tensor.matmul(
                                ps_s[:, _ts(nt, 512)],
                                kT_t[base : base + 64, _ts(mt, 128)],
                                qT_t[base : base + 64, _ts(nt, 512)],
                                start=True,
                                stop=True,
                            )
                        # probs_unnorm = exp(S^T/8 + mask[kpos]) in bf16
                        ex = expool.tile([128, S], BF16, tag="ex", name=f"ex{h}_{mt}")
                        nc.scalar.activation(
                            ex,
                            ps_s,
                            mybir.ActivationFunctionType.Exp,
                            bias=mask_sb[:, mt : mt + 1],
                            scale=0.125,
                        )
                        pending.append((h, mt, ex))
                        depth = 1 if (hp == HP - 1 and hl == 1) else 2
                        while len(pending) > depth:
                            emit_pv(*pending.pop(0))
                        # paced fillers: V nt0 blocks ride units 0-7 (each
                        # must precede PV(head0, mt) two units later); V nt1
                        # blocks (heads 8-15, first used at unit 64) spread
                        # over units 16..; QK projections at ~0.4 group/unit
                        # so late units still have PE work while ACT runs.
                        if unit < 8 or (unit >= 16 and unit % 3 == 1):
                            next(v_gen, "END")
                        qk_owed += 0.4
                        if qk_owed >= 1.0:
                            qk_owed -= 1.0
                            pull_filler(1)
                        unit += 1
            for args in pending:
                emit_pv(*args)
    nc.compile()
    return nc


_NC_CACHE = None


def _get_nc():
    global _NC_CACHE
    if _NC_CACHE is None:
        _NC_CACHE = build_program()
    return _NC_CACHE


def _prep_inputs(hidden_states, attention_mask, head_mask, Wq, bq, Wk, bk, Wv, bv):
    import ml_dtypes

    bf16 = ml_dtypes.bfloat16
    qk_np = bf16
    hidden_states = np.asarray(hidden_states, dtype=np.float32)
    attention_mask = np.asarray(attention_mask, dtype=np.float32)
    head_mask = np.asarray(head_mask, dtype=np.float32)
    Wq = np.asarray(Wq, dtype=np.float32)
    bq = np.asarray(bq, dtype=np.float32)
    Wk = np.asarray(Wk, dtype=np.float32)
    bk = np.asarray(bk, dtype=np.float32)
    Wv = np.asarray(Wv, dtype=np.float32)
    bv = np.asarray(bv, dtype=np.float32)

    # fold head_mask into Wv/bv (probs*hm @ V == probs @ (hm*V)); the
    # denominator comes from the raw exp values so it stays unscaled.
    hm = head_mask.reshape(H)
    hscale = np.repeat(hm, DH).astype(np.float32)
    wqT = np.ascontiguousarray(Wq.T.astype(qk_np))
    wkT = np.ascontiguousarray(Wk.T.astype(qk_np))
    wvT = np.ascontiguousarray((Wv * hscale[:, None]).T.astype(bf16))
    bq2d = np.ascontiguousarray(bq.reshape(KT, 128).T)
    bk2d = np.ascontiguousarray(bk.reshape(KT, 128).T)
    bvrow = (bv * hscale).reshape(1, D)

    mask = np.broadcast_to(
        attention_mask.reshape(attention_mask.shape[0], -1)[:, -S:], (N_CORES, S)
    )

    in_maps = []
    for b in range(N_CORES):
        hTf = hidden_states[b].T
        in_maps.append(
            {
                "hTb": np.ascontiguousarray(hTf.astype(bf16)),
                "wqT": wqT,
                "wkT": wkT,
                "wvT": wvT,
                "bq2d": bq2d,
                "bk2d": bk2d,
                "bvrow": bvrow,
                "mask2d": np.ascontiguousarray(mask[b].reshape(KT, 128).T),
            }
        )
    return in_maps


def _install_trace_shim():
    """antenv.axon_hooks is absent in this image; provide it so trace=True works."""
    import types

    if "antenv.axon_hooks" in sys.modules:
        return
    mod = types.ModuleType("antenv.axon_hooks")
    mod._hook = None

    def _set(h):
        mod._hook = h

    def _get():
        return mod._hook

    mod.set_axon_ntff_profile_hook = _set
    mod.get_axon_ntff_profile_hook = _get
    sys.modules["antenv.axon_hooks"] = mod
    try:
        from trn_agent_boot.trn_boot import _ntff_profile_via_ctypes

        _set(_ntff_profile_via_ctypes("/opt/axon/libaxon_pjrt.so"))
    except Exception:
        pass


def _kernel_impl(trace=False, **inputs):
    nc = _get_nc()
    in_maps = _prep_inputs(**inputs)
    kwargs = {}
    if trace:
        _install_trace_shim()
        kwargs["trace"] = True
        kwargs["trace_cores"] = list(range(N_CORES))
    res = run_bass_kernel_spmd(nc, in_maps, core_ids=list(range(N_CORES)), **kwargs)
    out = np.empty((N_CORES, S, D), dtype=np.float32)
    for b in range(N_CORES):
        ctxu = np.asarray(res.results[b]["ctxT"], dtype=np.float32)
        denf = np.asarray(res.results[b]["dens"], dtype=np.float32).reshape(H, S)
        out[b] = (ctxu / np.repeat(denf, DH, axis=0)).T
    return out, res


def kernel(**inputs) -> np.ndarray:
    return _kernel_impl(trace=False, **inputs)[0]


# revision 10
# speedup vs baseline: 1.2082x; 1.0024x over previous
"""MoEBertSelfAttention on 8 Trainium2 NeuronCores.

Strategy: data-parallel over batch (B=8 -> one batch element per core).
Each core computes its element's full self-attention.

v3 design:
  - on-device dataflow fully transposed (no on-chip transposes): host passes
    H^T / W^T; scores are computed as S^T (key position on partitions) so
    the additive attention mask is a per-partition bias on the exp()
    activation,
  - all matmul operands bf16 (fp32 PSUM), except the Q/K projections which
    run in fp8(e4m3) with perf_mode=DoubleRow: two 128-deep contraction
    chunks packed per matmul -> ~1.5x projection throughput. The fp8
    rounding only perturbs attention logits (~4% of their unit-scale std);
    the softmax ratio cancels most of the downstream effect,
  - V projection stays bf16 (its rounding lands directly in the output),
  - the softmax denominator rides as an extra all-ones bf16 column of V in
    the PV matmul ([64 cols of V_h | ones] per head),
  - normalization happens on the host: the kernel returns unnormalized
    ctx^T plus the denominator rows; the host divides. Numerator and
    denominator use the same rounded exp values, so the softmax ratio is
    exact up to fp32 accumulation,
  - head_mask folded into Wv/bv on the host (exact).

Pipelining: per (head, kpos-chunk) unit the PE runs two 512-wide score
matmuls; the PV matmuls of older units plus projection psum-groups for the
next head pair / V blocks fill the PE while ACT runs exp(), keeping the PE
queue dense so it stays at the 2.4 GHz p-state.
"""

import sys

if "/opt/trn_rl_repo" not in sys.path:
    sys.path.insert(0, "/opt/trn_rl_repo")

import numpy as np

import concourse.bacc as bacc
import concourse.bass as bass
import concourse.tile as tile
from concourse import mybir
from concourse.bass_utils import run_bass_kernel_spmd

S = 1024  # sequence length
D = 1024  # hidden size
H = 16  # heads
DH = 64  # head size
KT = D // 128  # 128-row tiles along a feature dim
NT = S // 512  # 512-col tiles along the sequence
HP = H // 2  # head pairs
N_CORES = 8

F32 = mybir.dt.float32
BF16 = mybir.dt.bfloat16
FP8 = mybir.dt.float8e4

QK_FP8 = False  # fp8 Q/K projections: measured rel err 4.8e-2 > 2e-2 gate


def _ts(i, n):
    return slice(i * n, (i + 1) * n)


def build_program():
    nc = bacc.Bacc("TRN2", target_bir_lowering=False, debug=False, num_devices=N_CORES)

    qk_dt = BF16
    hTb = nc.dram_tensor("hTb", [D, S], BF16, kind="ExternalInput").ap()
    wqT = nc.dram_tensor("wqT", [D, D], qk_dt, kind="ExternalInput").ap()
    wkT = nc.dram_tensor("wkT", [D, D], qk_dt, kind="ExternalInput").ap()
    wvT = nc.dram_tensor("wvT", [D, D], BF16, kind="ExternalInput").ap()
    bq2d = nc.dram_tensor("bq2d", [128, KT], F32, kind="ExternalInput").ap()
    bk2d = nc.dram_tensor("bk2d", [128, KT], F32, kind="ExternalInput").ap()
    bvrow = nc.dram_tensor("bvrow", [1, D], F32, kind="ExternalInput").ap()
    mask2d = nc.dram_tensor("mask2d", [128, KT], F32, kind="ExternalInput").ap()
    ctxT = nc.dram_tensor("ctxT", [D, S], F32, kind="ExternalOutput").ap()
    dens = nc.dram_tensor("dens", [H, NT, 512], F32, kind="ExternalOutput").ap()

    hTb_r = hTb.rearrange("(kt p) s -> p kt s", p=128)
    wqT_r = wqT.rearrange("(kt p) o -> p kt o", p=128)
    wkT_r = wkT.rearrange("(kt p) o -> p kt o", p=128)
    wvT_r = wvT.rearrange("(kt p) o -> p kt o", p=128)

    with tile.TileContext(nc) as tc:
        with (
            tc.tile_pool(name="persist", bufs=1) as persist,
            tc.tile_pool(name="wpool", bufs=2) as wpool,
            tc.tile_pool(name="qkpool", bufs=2) as qkpool,
            tc.tile_pool(name="expool", bufs=6) as expool,
            tc.tile_pool(name="outpool", bufs=3) as outpool,
            tc.tile_pool(name="ps", bufs=1, space="PSUM") as ps,
        ):
            # ---- persistent SBUF ----
            # head pair 0's weights + the QK copy of hT lead the DMA queues
            wq0_blk = wpool.tile([128, KT, 128], qk_dt, tag="wq", name="wq0")
            nc.sync.dma_start(out=wq0_blk, in_=wqT_r[:, :, _ts(0, 128)])
            wk0_blk = wpool.tile([128, KT, 128], qk_dt, tag="wk", name="wk0")
            nc.sync.dma_start(out=wk0_blk, in_=wkT_r[:, :, _ts(0, 128)])
            bq_sb = persist.tile([128, KT], F32)
            nc.sync.dma_start(out=bq_sb, in_=bq2d)
            bk_sb = persist.tile([128, KT], F32)
            nc.sync.dma_start(out=bk_sb, in_=bk2d)
            mask_sb = persist.tile([128, KT], F32)
            nc.sync.dma_start(out=mask_sb, in_=mask2d)
            # bv broadcast to all partitions (partition-step-0 DMA from DRAM)
            bv_bc = persist.tile([128, D], F32)
            nc.sync.dma_start(
                out=bv_bc,
                in_=bass.AP(tensor=bvrow.tensor, offset=0, ap=[[0, 128], [1, D]]),
            )
            hb_sb = persist.tile([128, KT, S], BF16)
            for hh in range(2):
                for kt in range(KT):
                    nc.sync.dma_start(
                        out=hb_sb[:, kt, _ts(hh, 512)], in_=hTb_r[:, kt, _ts(hh, 512)]
                    )
            h8_sb = hb_sb
            wvT_sb = persist.tile([128, KT, D], BF16)
            for kt in range(KT):
                for hh in range(2):
                    nc.sync.dma_start(
                        out=wvT_sb[:, kt, _ts(hh, 512)], in_=wvT_r[:, kt, _ts(hh, 512)]
                    )

            ones_f = persist.tile([128, H], BF16)
            nc.vector.memset(ones_f, 1.0)

            # V in token-major layout, one 65-wide block per head
            # ([64 cols of V_h | ones]); the ones column yields the softmax
            # denominator for free during the PV matmul.
            v_sb = persist.tile([128, KT, H * (DH + 1)], BF16)
            v4 = v_sb.rearrange("p st (h c) -> p st h c", c=DH + 1)
            for st in range(KT):
                nc.vector.tensor_copy(
                    v4[:, st, :, DH : DH + 1],
                    ones_f.rearrange("p (h o) -> p h o", o=1),
                )

            qk_tiles = {}

            def emit_qk(hp, preloaded=None):
                """Q^T/K^T projection for head pair hp, yielded one
                (tensor, nt) psum-group at a time (group-atomic: the shared
                proj psum slot must not interleave two accumulation groups)."""
                if preloaded is not None:
                    wq_blk, wk_blk = preloaded
                else:
                    wq_blk = wpool.tile([128, KT, 128], qk_dt, tag="wq", name=f"wq{hp}")
                    nc.sync.dma_start(out=wq_blk, in_=wqT_r[:, :, _ts(hp, 128)])
                    wk_blk = wpool.tile([128, KT, 128], qk_dt, tag="wk", name=f"wk{hp}")
                    nc.sync.dma_start(out=wk_blk, in_=wkT_r[:, :, _ts(hp, 128)])
                res = []
                for pi, (blk, bias, tg) in enumerate(
                    ((wq_blk, bq_sb, "qT"), (wk_blk, bk_sb, "kT"))
                ):
                    t = qkpool.tile([128, S], BF16, tag=tg, name=f"{tg}{hp}")
                    for nt in range(NT):
                        p0 = ps.tile(
                            [128, 512], F32, tag="proj", bufs=1, name=f"pq{hp}_{pi}{nt}"
                        )
                        if QK_FP8:
                            # DoubleRow: two 128-deep contraction chunks per mm
                            for kk in range(KT // 2):
                                nc.tensor.matmul(
                                    p0,
                                    blk[:, 2 * kk : 2 * kk + 2, :],
                                    h8_sb[:, 2 * kk : 2 * kk + 2, _ts(nt, 512)],
                                    start=(kk == 0),
                                    stop=(kk == KT // 2 - 1),
                                    perf_mode=mybir.MatmulPerfMode.DoubleRow,
                                )
                        else:
                            for kt in range(KT):
                                nc.tensor.matmul(
                                    p0,
                                    blk[:, kt, :],
                                    h8_sb[:, kt, _ts(nt, 512)],
                                    start=(kt == 0),
                                    stop=(kt == KT - 1),
                                )
                        nc.vector.tensor_scalar_add(
                            t[:, _ts(nt, 512)], p0, bias[:, hp : hp + 1]
                        )
                        yield
                    res.append(t)
                qk_tiles[hp] = res

            def emit_v():
                """V projection, one (st, nt) psum-group per yield.
                V[s, o] = sum_d H^T[d, s] Wv^T[d, o] + bv[o]."""
                for nt in range(NT):
                    for st in range(KT):
                        ps_v = ps.tile(
                            [128, 512], F32, tag="projv", bufs=1, name=f"psv{st}_{nt}"
                        )
                        for kt in range(KT):
                            nc.tensor.matmul(
                                ps_v,
                                hb_sb[:, kt, _ts(st, 128)],
                                wvT_sb[:, kt, _ts(nt, 512)],
                                start=(kt == 0),
                                stop=(kt == KT - 1),
                            )
                        nc.vector.tensor_tensor(
                            out=v4[:, st, 8 * nt : 8 * nt + 8, 0:DH],
                            in0=ps_v.rearrange("p (h c) -> p h c", c=DH),
                            in1=bv_bc[:, _ts(nt, 512)].rearrange(
                                "p (h c) -> p h c", c=DH
                            ),
                            op=mybir.AluOpType.add,
                        )
                        yield

            # head pair 0's projections + first two V blocks up-front
            for _ in emit_qk(0, preloaded=(wq0_blk, wk0_blk)):
                pass

            # ---- attention ----
            pv_tiles = {}

            def emit_pv(h, mt, ex):
                if mt == 0:
                    pv_tiles[h] = [
                        ps.tile(
                            [DH + 1, 512], F32, tag="pv", bufs=2, name=f"pspv{h}_{i}"
                        )
                        for i in range(NT)
                    ]
                for nt in range(NT):
                    nc.tensor.matmul(
                        pv_tiles[h][nt],
                        v_sb[:, mt, h * (DH + 1) : (h + 1) * (DH + 1)],
                        ex[:, _ts(nt, 512)],
                        start=(mt == 0),
                        stop=(mt == KT - 1),
                    )
                if mt == KT - 1:
                    emit_out(h)

            def emit_out(h):
                # drain PSUM -> SBUF -> DRAM (ctx rows 0:64, denominator row 64)
                for nt in range(NT):
                    c_sb = outpool.tile(
                        [DH + 1, 512], F32, tag="ctx", name=f"c{h}_{nt}"
                    )
                    nc.vector.tensor_copy(c_sb, pv_tiles[h][nt])
                    nc.sync.dma_start(
                        out=ctxT[_ts(h, DH), _ts(nt, 512)], in_=c_sb[0:DH, :]
                    )
                    nc.sync.dma_start(out=dens[h, nt, :], in_=c_sb[DH : DH + 1, :])

            # filler: next-pair QK projections, then V blocks (group-atomic)
            pending = []
            fillers = []

            def pull_filler(n):
                cnt = 0
                while fillers and cnt < n:
                    if next(fillers[0], "END") == "END":
                        fillers.pop(0)
                    else:
                        cnt += 1

            v_gen = emit_v()
            unit = 0
            qk_owed = 0.0
            for hp in range(HP):
                qT_t, kT_t = qk_tiles[hp]
                if hp + 1 < HP:
                    fillers.append(emit_qk(hp + 1))
                for hl in range(2):
                    h = 2 * hp + hl
                    base = 64 * hl
                    for mt in range(KT):
                        # S^T[kpos, q] for this head
                        ps_s = ps.tile(
                            [128, 1024], F32, tag="sc", bufs=2, name=f"pss{h}_{mt}"
                        )
                        for nt in range(NT):
                            nc.tensor.matmul(
                                ps_s[:, _ts(nt, 512)],
                                kT_t[base : base + 64, _ts(mt, 128)],
                                qT_t[base : base + 64, _ts(nt, 512)],
                                start=True,
                                stop=True,
                            )
                        # probs_unnorm = exp(S^T/8 + mask[kpos]) in bf16
                        ex = expool.tile([128, S], BF16, tag="ex", name=f"ex{h}_{mt}")
                        nc.scalar.activation(
                            ex,
                            ps_s,
                            mybir.ActivationFunctionType.Exp,
                            bias=mask_sb[:, mt : mt + 1],
                            scale=0.125,
                        )
                        pending.append((h, mt, ex))
                        depth = 1 if (hp == HP - 1 and hl == 1) else 3
                        while len(pending) > depth:
                            emit_pv(*pending.pop(0))
                        # paced fillers: V nt0 blocks ride units 0-7 (each
                        # must precede PV(head0, mt) two units later); V nt1
                        # blocks (heads 8-15, first used at unit 64) spread
                        # over units 16..; QK projections at ~0.4 group/unit
                        # so late units still have PE work while ACT runs.
                        if unit < 8 or (unit >= 16 and unit % 3 == 1):
                            next(v_gen, "END")
                        qk_owed += 0.4
                        if qk_owed >= 1.0:
                            qk_owed -= 1.0
                            pull_filler(1)
                        unit += 1
            for args in pending:
                emit_pv(*args)
    nc.compile()
    return nc


_NC_CACHE = None


def _get_nc():
    global _NC_CACHE
    if _NC_CACHE is None:
        _NC_CACHE = build_program()
    return _NC_CACHE


def _prep_inputs(hidden_states, attention_mask, head_mask, Wq, bq, Wk, bk, Wv, bv):
    import ml_dtypes

    bf16 = ml_dtypes.bfloat16
    qk_np = bf16
    hidden_states = np.asarray(hidden_states, dtype=np.float32)
    attention_mask = np.asarray(attention_mask, dtype=np.float32)
    head_mask = np.asarray(head_mask, dtype=np.float32)
    Wq = np.asarray(Wq, dtype=np.float32)
    bq = np.asarray(bq, dtype=np.float32)
    Wk = np.asarray(Wk, dtype=np.float32)
    bk = np.asarray(bk, dtype=np.float32)
    Wv = np.asarray(Wv, dtype=np.float32)
    bv = np.asarray(bv, dtype=np.float32)

    # fold head_mask into Wv/bv (probs*hm @ V == probs @ (hm*V)); the
    # denominator comes from the raw exp values so it stays unscaled.
    hm = head_mask.reshape(H)
    hscale = np.repeat(hm, DH).astype(np.float32)
    wqT = np.ascontiguousarray(Wq.T.astype(qk_np))
    wkT = np.ascontiguousarray(Wk.T.astype(qk_np))
    wvT = np.ascontiguousarray((Wv * hscale[:, None]).T.astype(bf16))
    bq2d = np.ascontiguousarray(bq.reshape(KT, 128).T)
    bk2d = np.ascontiguousarray(bk.reshape(KT, 128).T)
    bvrow = (bv * hscale).reshape(1, D)

    mask = np.broadcast_to(
        attention_mask.reshape(attention_mask.shape[0], -1)[:, -S:], (N_CORES, S)
    )

    in_maps = []
    for b in range(N_CORES):
        hTf = hidden_states[b].T
        in_maps.append(
            {
                "hTb": np.ascontiguousarray(hTf.astype(bf16)),
                "wqT": wqT,
                "wkT": wkT,
                "wvT": wvT,
                "bq2d": bq2d,
                "bk2d": bk2d,
                "bvrow": bvrow,
                "mask2d": np.ascontiguousarray(mask[b].reshape(KT, 128).T),
            }
        )
    return in_maps


def _install_trace_shim():
    """antenv.axon_hooks is absent in this image; provide it so trace=True works."""
    import types

    if "antenv.axon_hooks" in sys.modules:
        return
    mod = types.ModuleType("antenv.axon_hooks")
    mod._hook = None

    def _set(h):
        mod._hook = h

    def _get():
        return mod._hook

    mod.set_axon_ntff_profile_hook = _set
    mod.get_axon_ntff_profile_hook = _get
    sys.modules["antenv.axon_hooks"] = mod
    try:
        from trn_agent_boot.trn_boot import _ntff_profile_via_ctypes

        _set(_ntff_profile_via_ctypes("/opt/axon/libaxon_pjrt.so"))
    except Exception:
        pass


def _kernel_impl(trace=False, **inputs):
    nc = _get_nc()
    in_maps = _prep_inputs(**inputs)
    kwargs = {}
    if trace:
        _install_trace_shim()
        kwargs["trace"] = True
        kwargs["trace_cores"] = list(range(N_CORES))
    res = run_bass_kernel_spmd(nc, in_maps, core_ids=list(range(N_CORES)), **kwargs)
    out = np.empty((N_CORES, S, D), dtype=np.float32)
    for b in range(N_CORES):
        ctxu = np.asarray(res.results[b]["ctxT"], dtype=np.float32)
        denf = np.asarray(res.results[b]["dens"], dtype=np.float32).reshape(H, S)
        out[b] = (ctxu / np.repeat(denf, DH, axis=0)).T
    return out, res


def kernel(**inputs) -> np.ndarray:
    return _kernel_impl(trace=False, **inputs)[0]
